# revision 13
# baseline (speedup 1.0000x reference)
"""Multi-head self-attention (dense transformer block) on 8 Trainium2 cores.

Tensor-parallel over heads: core m handles heads {2m, 2m+1} for both batch
elements. The reference's RoPE uses angles that depend only on the head
index (not the position), so it is a fixed orthogonal rotation per head;
we fold it (and the 1/sqrt(D) score scale) into the QKV weights on the
host. Device pipeline per core:

  1. qT/kT/vT = (w_slice)^T @ x^T          [d-major, tokens on free dim]
  2. V~ = transpose(vT) with a ones column appended per head
  3. per (batch, head): ST = k^T q  -> exp -> PV matmul with V~ gives
     both the output numerator and the softmax denominator (ones col)
  4. normalize via reciprocal + gpsimd partition_broadcast + DVE mult
  5. AllGather the per-core head outputs (hidden^T) per token chunk
  6. column-parallel FC per chunk: out^T slice = w_fc_slice^T @ hidden^T

All matmuls run in float32r (fp32 data, fast PE mode, ~1e-4 rel err).
"""

import numpy as np

import concourse.bass as bass
import concourse.mybir as mybir
from concourse.bass_utils import run_bass_kernel_spmd
from concourse.tile import TileContext

# Problem shapes (hardcoded per contract)
B, T, C = 2, 2048, 1024
H, D = 16, 64
N_CORES = 8
HPC = H // N_CORES          # heads per core = 2
HB = HPC * D                # head-block width per core = 128
NT = B * T                  # 4096 tokens
P = 128
TCH = 512                   # token chunk (matmul free dim)
F32 = mybir.dt.float32
F32R = mybir.dt.float32r


def _rope_mats():
    """Per-head [D, D] matrices Rt with q_roped_row = q_row @ Rt (row-vector
    convention), matching reference._rope where the angle is head-dependent
    and position-independent."""
    inv_freq = 1.0 / (10000.0 ** (np.arange(0, D, 2, dtype=np.float64) / D))
    mats = []
    for h in range(H):
        theta = h * inv_freq                      # [D/2]
        c, s = np.cos(theta), np.sin(theta)
        R = np.zeros((D, D), dtype=np.float64)
        R[::2, ::2] = np.diag(c)                  # even <- even*cos
        R[1::2, ::2] = -np.diag(s)                # even <- odd*(-sin)
        R[::2, 1::2] = np.diag(s)                 # odd  <- even*sin
        R[1::2, 1::2] = np.diag(c)                # odd  <- odd*cos
        mats.append(R)
    return mats


def split_sync_commands(nc, max_waits=1, max_updates=1):
    """This container's walrus supports only one sync wait / update per
    instruction. Split excess waits into preceding EventSemaphore instrs on
    the same engine queue, and excess updates into following ones."""
    n_split = 0
    for f in nc.m.functions:
        for bb in f.blocks:
            insts = list(bb.instructions)
            new_list = []
            changed = False
            for inst in insts:
                si = inst.sync_info
                waits = list(si.on_wait) if (si and si.on_wait) else []
                if len(waits) > max_waits:
                    for w in waits[max_waits:]:
                        ev = mybir.InstEventSemaphore(
                            name=f"{inst.name}-wsplit-{n_split}",
                            engine=inst.engine, ins=[], outs=[],
                            sync_info=mybir.SyncInfo(on_wait=[w], on_update=[]),
                        )
                        n_split += 1
                        new_list.append(ev)
                    si.on_wait = waits[:max_waits]
                    changed = True
                new_list.append(inst)
                updates = list(si.on_update) if (si and si.on_update) else []
                if len(updates) > max_updates:
                    opcode = type(inst).__name__
                    if "Dma" in opcode or "DMA" in opcode:
                        raise RuntimeError(
                            f"DMA inst {inst.name} has {len(updates)} updates")
                    si.on_update = updates[:max_updates]
                    for u in updates[max_updates:]:
                        ev = mybir.InstEventSemaphore(
                            name=f"{inst.name}-usplit-{n_split}",
                            engine=inst.engine, ins=[], outs=[],
                            sync_info=mybir.SyncInfo(on_wait=[], on_update=[u]),
                        )
                        n_split += 1
                        new_list.append(ev)
                    changed = True
            if changed:
                bb.instructions = new_list
    return n_split


def build_kernel():
    nc = bass.Bass(num_devices=N_CORES)

    xT = nc.dram_tensor("xT", [C, NT], F32R, kind="ExternalInput")
    wq = nc.dram_tensor("wq", [C, HB], F32R, kind="ExternalInput")
    wk = nc.dram_tensor("wk", [C, HB], F32R, kind="ExternalInput")
    wv = nc.dram_tensor("wv", [C, HB], F32R, kind="ExternalInput")
    bq = nc.dram_tensor("bq", [HB, 1], F32, kind="ExternalInput")
    bk = nc.dram_tensor("bk", [HB, 1], F32, kind="ExternalInput")
    bv = nc.dram_tensor("bv", [HB, 1], F32, kind="ExternalInput")
    wfc = nc.dram_tensor("wfc", [C, HB], F32R, kind="ExternalInput")
    bfc = nc.dram_tensor("bfc", [HB, 1], F32, kind="ExternalInput")
    ident_in = nc.dram_tensor("ident", [P, P], F32R, kind="ExternalInput")
    ones_in = nc.dram_tensor("ones", [P, D], F32R, kind="ExternalInput")
    # ones64 for the K=1 denominator-broadcast matmul
    outT = nc.dram_tensor("outT", [HB, NT], F32, kind="ExternalOutput")

    CB = C // P                      # 8 contraction blocks
    SBLK = T // P                    # 16 s-blocks per batch
    NBC = T // TCH                   # 4 chunks per batch
    Exp = mybir.ActivationFunctionType.Exp

    with TileContext(nc) as tc:
        with (
            tc.tile_pool(name="consts", bufs=1) as consts,
            tc.tile_pool(name="qkv", bufs=1) as qkvp,
            tc.tile_pool(name="work", bufs=2) as work,
            tc.tile_pool(name="expp", bufs=4) as expp,
            tc.tile_pool(name="psum", bufs=1, space="PSUM") as psum,
            tc.tile_pool(name="dram", bufs=1, space="DRAM") as dram,
        ):
            # ---- constants ----
            w_sb = {}
            for name, t in (("wq", wq), ("wk", wk), ("wv", wv), ("wfc", wfc)):
                w_t = consts.tile([P, CB, HB], F32R, name=f"{name}_sb")
                nc.sync.dma_start(
                    w_t[:], t[:, :].rearrange("(cb p) o -> p cb o", p=P))
                w_sb[name] = w_t
            b_sb = {}
            for name, t in (("bq", bq), ("bk", bk), ("bv", bv), ("bfc", bfc)):
                b_t = consts.tile([HB, 1], F32, name=f"{name}_sb")
                nc.sync.dma_start(b_t[:], t[:])
                b_sb[name] = b_t
            identity = consts.tile([P, P], F32R, name="identity")
            nc.sync.dma_start(identity[:], ident_in[:])
            ones64 = consts.tile([1, D], F32R, name="ones64")
            nc.sync.dma_start(ones64[:], ones_in[0:1, 0:D])

            # ---- persistent qkv storage ----
            qT = qkvp.tile([P, NT], F32R, name="qT")
            kz = [qkvp.tile([P, NT], F32R, name=f"kz{h}")
                  for h in range(HPC)]
            vT = qkvp.tile([P, NT], F32R, name="vT")
            # V~: [s-in-block, s-block, 2*(D+1)] with ones at cols D and 2D+1
            vtl = qkvp.tile([P, SBLK * B, 2 * (D + 1)], F32R, name="vtl")
            nc.sync.dma_start(vtl[:, :, D:D + 1],
                              ones_in[:, 0:SBLK * B, None])
            nc.sync.dma_start(vtl[:, :, 2 * D + 1:2 * D + 2],
                              ones_in[:, 0:SBLK * B, None])

            ag_in = {}
            ag_out = {}
            for b in range(B):
                ag_in[b] = dram.tile([HB, T], F32R, name=f"ag_in_{b}",
                                     tag=f"ag_in_{b}")
                ag_out[b] = dram.tile([N_CORES * HB, T], F32R,
                                      name=f"ag_out_{b}", tag=f"ag_out_{b}",
                                      addr_space="Shared")

            def proj_chunk(tcix):
                tsl = slice(tcix * TCH, (tcix + 1) * TCH)
                xt_tiles = []
                for cb in range(CB):
                    xt = work.tile([P, TCH], F32R, tag="xt", name=f"xt_{cb}",
                                   bufs=2 * CB)
                    nc.sync.dma_start(xt[:], xT[cb * P:(cb + 1) * P, tsl])
                    xt_tiles.append(xt)
                for wname, bname, dst in (("wq", "bq", qT), ("wk", "bk", None),
                                          ("wv", "bv", vT)):
                    ps = psum.tile([P, TCH], F32, tag="mm", name="ps_proj",
                                   bufs=3)
                    for cb in range(CB):
                        nc.tensor.matmul(ps[:], w_sb[wname][:, cb, :],
                                         xt_tiles[cb],
                                         start=(cb == 0), stop=(cb == CB - 1))
                    if dst is not None:
                        nc.vector.tensor_scalar_add(dst[:, tsl], ps[:],
                                                    b_sb[bname][:])
                    else:
                        # k: write zero-padded per-head copies for full-K ST
                        nc.vector.tensor_scalar_add(
                            kz[0][0:D, tsl], ps[0:D, :], b_sb["bk"][0:D])
                        nc.vector.tensor_scalar_mul(
                            kz[0][D:P, tsl], ps[D:P, :], 0.0)
                        nc.vector.tensor_scalar_add(
                            kz[1][D:P, tsl], ps[D:P, :], b_sb["bk"][D:P])
                        nc.vector.tensor_scalar_mul(
                            kz[1][0:D, tsl], ps[0:D, :], 0.0)

            def vtl_block(sb):
                pst = psum.tile([P, P], F32R, tag="mmtr", name="ps_tr",
                                bufs=1)
                nc.tensor.transpose(pst[:], vT[:, sb * P:(sb + 1) * P],
                                    identity[:])
                nc.vector.tensor_copy(out=vtl[:, sb, 0:D], in_=pst[:, 0:D])
                nc.vector.tensor_copy(out=vtl[:, sb, D + 1:2 * D + 1],
                                      in_=pst[:, D:2 * D])

            def attention_chunk(b, tcix):
                tsl = slice(b * T + tcix * TCH, b * T + (tcix + 1) * TCH)
                pv_ps = [
                    psum.tile([P, TCH], F32, tag=f"pv{h}",
                              name=f"ps_pv{h}", bufs=2)
                    for h in range(HPC)
                ]
                for sb in range(SBLK):
                    gsb = b * SBLK + sb
                    ssl = slice(b * T + sb * P, b * T + sb * P + P)
                    e_tiles = []
                    for h in range(HPC):
                        ps_st = psum.tile([P, TCH], F32, tag="mm",
                                          name="ps_st", bufs=3)
                        nc.tensor.matmul(ps_st[:], kz[h][:, ssl],
                                         qT[:, tsl], start=True, stop=True)
                        e = expp.tile([P, TCH], F32R, tag=f"e{h}",
                                      name=f"e{h}", bufs=4)
                        nc.scalar.activation(e[:], ps_st[:], Exp)
                        e_tiles.append(e)
                    for h in range(HPC):
                        nc.tensor.matmul(
                            pv_ps[h][0:D + 1, :],
                            vtl[:, gsb, h * (D + 1):(h + 1) * (D + 1)],
                            e_tiles[h][:],
                            start=(sb == 0), stop=(sb == SBLK - 1))
                for h in range(HPC):
                    recip = work.tile([1, TCH], F32R, tag="recip",
                                      name="recip", bufs=2)
                    with nc.allow_low_precision(
                            reason="f32r==f32 bits; PE fast mode"):
                        nc.vector.reciprocal(recip[:], pv_ps[h][D:D + 1, :])
                    ps_bc = psum.tile([D, TCH], F32, tag="mmtr",
                                      name="ps_bc", bufs=1)
                    nc.tensor.matmul(ps_bc[:], ones64[:], recip[:],
                                     start=True, stop=True)
                    bc_sb = work.tile([D, TCH], F32R, tag="bc",
                                      name="bc_sb", bufs=2)
                    nc.vector.tensor_copy(out=bc_sb[:], in_=ps_bc[:])
                    ot = work.tile([D, TCH], F32R, tag="ot", name="ot",
                                   bufs=2)
                    nc.vector.tensor_mul(out=ot[:], in0=pv_ps[h][0:D, :],
                                         in1=bc_sb[:])
                    otsl = slice(tcix * TCH, (tcix + 1) * TCH)
                    nc.sync.dma_start(
                        ag_in[b][h * D:(h + 1) * D, otsl], ot[:])

            def allgather(b):
                nc.gpsimd.collective_compute(
                    "AllGather", mybir.AluOpType.bypass,
                    replica_groups=[list(range(N_CORES))],
                    ins=[ag_in[b][:].opt()],
                    outs=[ag_out[b][:].opt()])

            def fc_chunk(b, tcix):
                tsl = slice(b * T + tcix * TCH, b * T + (tcix + 1) * TCH)
                otsl = slice(tcix * TCH, (tcix + 1) * TCH)
                z_tiles = []
                for cb in range(CB):
                    z = work.tile([P, TCH], F32R, tag="z", name=f"z_{cb}",
                                  bufs=2 * CB)
                    nc.gpsimd.dma_start(
                        z[:], ag_out[b][cb * P:(cb + 1) * P, otsl])
                    z_tiles.append(z)
                ps = psum.tile([P, TCH], F32, tag="mm", name="ps_fc",
                               bufs=3)
                for cb in range(CB):
                    nc.tensor.matmul(ps[:], w_sb["wfc"][:, cb, :],
                                     z_tiles[cb],
                                     start=(cb == 0), stop=(cb == CB - 1))
                osb = work.tile([P, TCH], F32, tag="osb", name="osb", bufs=2)
                nc.vector.tensor_scalar_add(osb[:], ps[:], b_sb["bfc"][:])
                nc.gpsimd.dma_start(outT[:, tsl], osb[:])

            # emission order == scheduler priority. proj/FC matmuls act as
            # PE gap-fillers during the exp-bound attention phases.
            for tcix in range(NBC):
                proj_chunk(tcix)
            for sb in range(SBLK):
                vtl_block(sb)
            for tcix in range(NBC):
                attention_chunk(0, tcix)
            allgather(0)
            for tcix in range(NBC):
                fc_chunk(0, tcix)
            for tcix in range(NBC, 2 * NBC):
                proj_chunk(tcix)
            for sb in range(SBLK, 2 * SBLK):
                vtl_block(sb)
            for tcix in range(NBC):
                attention_chunk(1, tcix)
            allgather(1)
            for tcix in range(NBC):
                fc_chunk(1, tcix)

    split_sync_commands(nc)
    return nc


_CACHE = {}


def _prep_inputs(x, w_qkv, b_qkv, w_fc, b_fc):
    """Host-side: fold RoPE + scale into weights, shard per core."""
    rope = _rope_mats()
    w_qkv = np.asarray(w_qkv, dtype=np.float64)
    b_qkv = np.asarray(b_qkv, dtype=np.float64)
    wq_f = w_qkv[:, 0:C].copy()
    wk_f = w_qkv[:, C:2 * C].copy()
    wv_f = w_qkv[:, 2 * C:3 * C].copy()
    bq_f = b_qkv[0:C].copy()
    bk_f = b_qkv[C:2 * C].copy()
    bv_f = b_qkv[2 * C:3 * C].copy()
    scale = 1.0 / np.sqrt(D)
    for h in range(H):
        sl = slice(h * D, (h + 1) * D)
        wq_f[:, sl] = (wq_f[:, sl] @ rope[h]) * scale
        bq_f[sl] = (bq_f[sl] @ rope[h]) * scale
        wk_f[:, sl] = wk_f[:, sl] @ rope[h]
        bk_f[sl] = bk_f[sl] @ rope[h]

    xT = np.ascontiguousarray(
        np.asarray(x, dtype=np.float32).reshape(NT, C).T)

    in_maps = []
    for m in range(N_CORES):
        sl = slice(m * HB, (m + 1) * HB)
        in_maps.append({
            "xT": xT,
            "wq": np.ascontiguousarray(wq_f[:, sl], dtype=np.float32),
            "wk": np.ascontiguousarray(wk_f[:, sl], dtype=np.float32),
            "wv": np.ascontiguousarray(wv_f[:, sl], dtype=np.float32),
            "bq": np.ascontiguousarray(bq_f[sl, None], dtype=np.float32),
            "bk": np.ascontiguousarray(bk_f[sl, None], dtype=np.float32),
            "bv": np.ascontiguousarray(bv_f[sl, None], dtype=np.float32),
            "wfc": np.ascontiguousarray(w_fc[:, sl], dtype=np.float32),
            "bfc": np.ascontiguousarray(
                np.asarray(b_fc, dtype=np.float32)[sl, None]),
            "ident": np.eye(P, dtype=np.float32),
            "ones": np.ones((P, D), dtype=np.float32),
        })
    return in_maps


def kernel(x, w_qkv, b_qkv, w_fc, b_fc, _trace=False):
    in_maps = _prep_inputs(x, w_qkv, b_qkv, w_fc, b_fc)
    if "nc" not in _CACHE:
        _CACHE["nc"] = build_kernel()
    nc = _CACHE["nc"]
    res = run_bass_kernel_spmd(nc, in_maps, core_ids=list(range(N_CORES)),
                               trace=_trace)
    _CACHE["last_result"] = res
    out = np.concatenate(
        [res.results[m]["outT"].T for m in range(N_CORES)], axis=1)
    return np.ascontiguousarray(out.reshape(B, T, C))


# revision 14
# speedup vs baseline: 1.1622x; 1.1622x over previous
"""Multi-head self-attention (dense transformer block) on 8 Trainium2 cores.

Tensor-parallel over heads: core m handles heads {2m, 2m+1} for both batch
elements. The reference's RoPE uses angles that depend only on the head
index (not the position), so it is a fixed orthogonal rotation per head;
we fold it (and the 1/sqrt(D) score scale) into the QKV weights on the
host. Device pipeline per core:

  1. qT/kT/vT = (w_slice)^T @ x^T          [d-major, tokens on free dim]
  2. V~ = transpose(vT) with a ones column appended per head
  3. per (batch, head): ST = k^T q  -> exp -> PV matmul with V~ gives
     both the output numerator and the softmax denominator (ones col)
  4. normalize via reciprocal + gpsimd partition_broadcast + DVE mult
  5. AllGather the per-core head outputs (hidden^T) per token chunk
  6. column-parallel FC per chunk: out^T slice = w_fc_slice^T @ hidden^T

All matmuls run in float32r (fp32 data, fast PE mode, ~1e-4 rel err).
"""

import numpy as np

import concourse.bass as bass
import concourse.mybir as mybir
from concourse.bass_utils import run_bass_kernel_spmd
from concourse.tile import TileContext

# Problem shapes (hardcoded per contract)
B, T, C = 2, 2048, 1024
H, D = 16, 64
N_CORES = 8
HPC = H // N_CORES          # heads per core = 2
HB = HPC * D                # head-block width per core = 128
NT = B * T                  # 4096 tokens
P = 128
TCH = 512                   # token chunk (matmul free dim)
F32 = mybir.dt.float32
F32R = mybir.dt.float32r


def _rope_mats():
    """Per-head [D, D] matrices Rt with q_roped_row = q_row @ Rt (row-vector
    convention), matching reference._rope where the angle is head-dependent
    and position-independent."""
    inv_freq = 1.0 / (10000.0 ** (np.arange(0, D, 2, dtype=np.float64) / D))
    mats = []
    for h in range(H):
        theta = h * inv_freq                      # [D/2]
        c, s = np.cos(theta), np.sin(theta)
        R = np.zeros((D, D), dtype=np.float64)
        R[::2, ::2] = np.diag(c)                  # even <- even*cos
        R[1::2, ::2] = -np.diag(s)                # even <- odd*(-sin)
        R[::2, 1::2] = np.diag(s)                 # odd  <- even*sin
        R[1::2, 1::2] = np.diag(c)                # odd  <- odd*cos
        mats.append(R)
    return mats


def split_sync_commands(nc, max_waits=1, max_updates=1):
    """This container's walrus supports only one sync wait / update per
    instruction. Split excess waits into preceding EventSemaphore instrs on
    the same engine queue, and excess updates into following ones."""
    n_split = 0
    for f in nc.m.functions:
        for bb in f.blocks:
            insts = list(bb.instructions)
            new_list = []
            changed = False
            for inst in insts:
                si = inst.sync_info
                waits = list(si.on_wait) if (si and si.on_wait) else []
                if len(waits) > max_waits:
                    for w in waits[max_waits:]:
                        ev = mybir.InstEventSemaphore(
                            name=f"{inst.name}-wsplit-{n_split}",
                            engine=inst.engine, ins=[], outs=[],
                            sync_info=mybir.SyncInfo(on_wait=[w], on_update=[]),
                        )
                        n_split += 1
                        new_list.append(ev)
                    si.on_wait = waits[:max_waits]
                    changed = True
                new_list.append(inst)
                updates = list(si.on_update) if (si and si.on_update) else []
                if len(updates) > max_updates:
                    opcode = type(inst).__name__
                    if "Dma" in opcode or "DMA" in opcode:
                        raise RuntimeError(
                            f"DMA inst {inst.name} has {len(updates)} updates")
                    si.on_update = updates[:max_updates]
                    for u in updates[max_updates:]:
                        ev = mybir.InstEventSemaphore(
                            name=f"{inst.name}-usplit-{n_split}",
                            engine=inst.engine, ins=[], outs=[],
                            sync_info=mybir.SyncInfo(on_wait=[], on_update=[u]),
                        )
                        n_split += 1
                        new_list.append(ev)
                    changed = True
            if changed:
                bb.instructions = new_list
    return n_split


def build_kernel():
    nc = bass.Bass(num_devices=N_CORES)

    xT = nc.dram_tensor("xT", [C, NT], F32R, kind="ExternalInput")
    wq = nc.dram_tensor("wq", [C, HB], F32R, kind="ExternalInput")
    wk = nc.dram_tensor("wk", [C, HB], F32R, kind="ExternalInput")
    wv = nc.dram_tensor("wv", [C, HB], F32R, kind="ExternalInput")
    bq = nc.dram_tensor("bq", [HB, 1], F32, kind="ExternalInput")
    bk = nc.dram_tensor("bk", [HB, 1], F32, kind="ExternalInput")
    bv = nc.dram_tensor("bv", [HB, 1], F32, kind="ExternalInput")
    wfc = nc.dram_tensor("wfc", [C, HB], F32R, kind="ExternalInput")
    bfc = nc.dram_tensor("bfc", [HB, 1], F32, kind="ExternalInput")
    ident_in = nc.dram_tensor("ident", [P, P], F32R, kind="ExternalInput")
    ones_in = nc.dram_tensor("ones", [P, D], F32R, kind="ExternalInput")
    # ones64 for the K=1 denominator-broadcast matmul
    outT = nc.dram_tensor("outT", [HB, NT], F32, kind="ExternalOutput")

    CB = C // P                      # 8 contraction blocks
    SBLK = T // P                    # 16 s-blocks per batch
    NBC = T // TCH                   # 4 chunks per batch
    Exp = mybir.ActivationFunctionType.Exp

    with TileContext(nc) as tc:
        with (
            tc.tile_pool(name="consts", bufs=1) as consts,
            tc.tile_pool(name="qkv", bufs=1) as qkvp,
            tc.tile_pool(name="work", bufs=2) as work,
            tc.tile_pool(name="expp", bufs=4) as expp,
            tc.tile_pool(name="psum", bufs=1, space="PSUM") as psum,
            tc.tile_pool(name="dram", bufs=1, space="DRAM") as dram,
        ):
            # ---- constants ----
            w_sb = {}
            for name, t in (("wq", wq), ("wk", wk), ("wv", wv), ("wfc", wfc)):
                w_t = consts.tile([P, CB, HB], F32R, name=f"{name}_sb")
                nc.sync.dma_start(
                    w_t[:], t[:, :].rearrange("(cb p) o -> p cb o", p=P))
                w_sb[name] = w_t
            b_sb = {}
            for name, t in (("bq", bq), ("bk", bk), ("bv", bv), ("bfc", bfc)):
                b_t = consts.tile([HB, 1], F32, name=f"{name}_sb")
                nc.sync.dma_start(b_t[:], t[:])
                b_sb[name] = b_t
            identity = consts.tile([P, P], F32R, name="identity")
            nc.sync.dma_start(identity[:], ident_in[:])
            ones64 = consts.tile([1, D], F32R, name="ones64")
            nc.sync.dma_start(ones64[:], ones_in[0:1, 0:D])

            # ---- persistent qkv storage ----
            qT = qkvp.tile([P, NT], F32R, name="qT")
            kz = [qkvp.tile([P, NT], F32R, name=f"kz{h}")
                  for h in range(HPC)]
            vT = qkvp.tile([P, NT], F32R, name="vT")
            # V~: [s-in-block, s-block, 2*(D+1)] with ones at cols D and 2D+1
            vtl = qkvp.tile([P, SBLK * B, 2 * (D + 1)], F32R, name="vtl")
            nc.sync.dma_start(vtl[:, :, D:D + 1],
                              ones_in[:, 0:SBLK * B, None])
            nc.sync.dma_start(vtl[:, :, 2 * D + 1:2 * D + 2],
                              ones_in[:, 0:SBLK * B, None])

            ag_in = {}
            ag_out = {}
            for b in range(B):
                ag_in[b] = dram.tile([HB, T], F32R, name=f"ag_in_{b}",
                                     tag=f"ag_in_{b}")
                ag_out[b] = dram.tile([N_CORES * HB, T], F32R,
                                      name=f"ag_out_{b}", tag=f"ag_out_{b}",
                                      addr_space="Shared")

            def proj_chunk(tcix):
                tsl = slice(tcix * TCH, (tcix + 1) * TCH)
                xt_tiles = []
                for cb in range(CB):
                    xt = work.tile([P, TCH], F32R, tag="xt", name=f"xt_{cb}",
                                   bufs=2 * CB)
                    nc.sync.dma_start(xt[:], xT[cb * P:(cb + 1) * P, tsl])
                    xt_tiles.append(xt)
                for wname, bname, dst in (("wq", "bq", qT), ("wk", "bk", None),
                                          ("wv", "bv", vT)):
                    ps = psum.tile([P, TCH], F32, tag="mm", name="ps_proj",
                                   bufs=3)
                    for cb in range(CB):
                        nc.tensor.matmul(ps[:], w_sb[wname][:, cb, :],
                                         xt_tiles[cb],
                                         start=(cb == 0), stop=(cb == CB - 1))
                    if dst is not None:
                        nc.vector.tensor_scalar_add(dst[:, tsl], ps[:],
                                                    b_sb[bname][:])
                    else:
                        # k: write zero-padded per-head copies for full-K ST
                        nc.vector.tensor_scalar_add(
                            kz[0][0:D, tsl], ps[0:D, :], b_sb["bk"][0:D])
                        nc.vector.tensor_scalar_mul(
                            kz[0][D:P, tsl], ps[D:P, :], 0.0)
                        nc.vector.tensor_scalar_add(
                            kz[1][D:P, tsl], ps[D:P, :], b_sb["bk"][D:P])
                        nc.vector.tensor_scalar_mul(
                            kz[1][0:D, tsl], ps[0:D, :], 0.0)

            def vtl_block(sb):
                pst = psum.tile([P, P], F32R, tag="mmtr", name="ps_tr",
                                bufs=1)
                nc.tensor.transpose(pst[:], vT[:, sb * P:(sb + 1) * P],
                                    identity[:])
                nc.vector.tensor_copy(out=vtl[:, sb, 0:D], in_=pst[:, 0:D])
                nc.vector.tensor_copy(out=vtl[:, sb, D + 1:2 * D + 1],
                                      in_=pst[:, D:2 * D])

            def attention_chunk(b, tcix):
                tsl = slice(b * T + tcix * TCH, b * T + (tcix + 1) * TCH)
                pv_ps = [
                    psum.tile([P, TCH], F32, tag=f"pv{h}",
                              name=f"ps_pv{h}", bufs=2)
                    for h in range(HPC)
                ]
                for sb in range(SBLK):
                    gsb = b * SBLK + sb
                    ssl = slice(b * T + sb * P, b * T + sb * P + P)
                    e_tiles = []
                    for h in range(HPC):
                        ps_st = psum.tile([P, TCH], F32, tag="mm",
                                          name="ps_st", bufs=3)
                        nc.tensor.matmul(ps_st[:], kz[h][:, ssl],
                                         qT[:, tsl], start=True, stop=True)
                        e = expp.tile([P, TCH], F32R, tag=f"e{h}",
                                      name=f"e{h}", bufs=4)
                        nc.scalar.activation(e[:], ps_st[:], Exp)
                        e_tiles.append(e)
                    for h in range(HPC):
                        nc.tensor.matmul(
                            pv_ps[h][0:D + 1, :],
                            vtl[:, gsb, h * (D + 1):(h + 1) * (D + 1)],
                            e_tiles[h][:],
                            start=(sb == 0), stop=(sb == SBLK - 1))
                for h in range(HPC):
                    recip = work.tile([1, TCH], F32R, tag="recip",
                                      name="recip", bufs=2)
                    with nc.allow_low_precision(
                            reason="f32r==f32 bits; PE fast mode"):
                        nc.vector.reciprocal(recip[:], pv_ps[h][D:D + 1, :])
                    ps_bc = psum.tile([D, TCH], F32, tag="mmtr",
                                      name="ps_bc", bufs=1)
                    nc.tensor.matmul(ps_bc[:], ones64[:], recip[:],
                                     start=True, stop=True)
                    bc_sb = work.tile([D, TCH], F32R, tag="bc",
                                      name="bc_sb", bufs=2)
                    nc.vector.tensor_copy(out=bc_sb[:], in_=ps_bc[:])
                    ot = work.tile([D, TCH], F32R, tag="ot", name="ot",
                                   bufs=2)
                    nc.vector.tensor_mul(out=ot[:], in0=pv_ps[h][0:D, :],
                                         in1=bc_sb[:])
                    otsl = slice(tcix * TCH, (tcix + 1) * TCH)
                    nc.sync.dma_start(
                        ag_in[b][h * D:(h + 1) * D, otsl], ot[:])

            def allgather(b):
                nc.gpsimd.collective_compute(
                    "AllGather", mybir.AluOpType.bypass,
                    replica_groups=[list(range(N_CORES))],
                    ins=[ag_in[b][:].opt()],
                    outs=[ag_out[b][:].opt()])

            def fc_chunk(b, tcix):
                tsl = slice(b * T + tcix * TCH, b * T + (tcix + 1) * TCH)
                otsl = slice(tcix * TCH, (tcix + 1) * TCH)
                z_tiles = []
                for cb in range(CB):
                    z = work.tile([P, TCH], F32R, tag="z", name=f"z_{cb}",
                                  bufs=2 * CB)
                    nc.gpsimd.dma_start(
                        z[:], ag_out[b][cb * P:(cb + 1) * P, otsl])
                    z_tiles.append(z)
                ps = psum.tile([P, TCH], F32, tag="mm", name="ps_fc",
                               bufs=3)
                for cb in range(CB):
                    nc.tensor.matmul(ps[:], w_sb["wfc"][:, cb, :],
                                     z_tiles[cb],
                                     start=(cb == 0), stop=(cb == CB - 1))
                osb = work.tile([P, TCH], F32, tag="osb", name="osb", bufs=2)
                nc.vector.tensor_scalar_add(osb[:], ps[:], b_sb["bfc"][:])
                nc.gpsimd.dma_start(outT[:, tsl], osb[:])

            # emission order == scheduler priority. proj/FC matmuls act as
            # PE gap-fillers during the exp-bound attention phases.
            for tcix in range(NBC):
                proj_chunk(tcix)
            for sb in range(SBLK):
                vtl_block(sb)
            for tcix in range(NBC):
                attention_chunk(0, tcix)
            allgather(0)
            for tcix in range(NBC, 2 * NBC):
                proj_chunk(tcix)
            for sb in range(SBLK, 2 * SBLK):
                vtl_block(sb)
            for tcix in range(NBC):
                attention_chunk(1, tcix)
                fc_chunk(0, tcix)
            allgather(1)
            for tcix in range(NBC):
                fc_chunk(1, tcix)

    split_sync_commands(nc)
    return nc


_CACHE = {}


def _prep_inputs(x, w_qkv, b_qkv, w_fc, b_fc):
    """Host-side: fold RoPE + scale into weights, shard per core."""
    rope = _rope_mats()
    w_qkv = np.asarray(w_qkv, dtype=np.float64)
    b_qkv = np.asarray(b_qkv, dtype=np.float64)
    wq_f = w_qkv[:, 0:C].copy()
    wk_f = w_qkv[:, C:2 * C].copy()
    wv_f = w_qkv[:, 2 * C:3 * C].copy()
    bq_f = b_qkv[0:C].copy()
    bk_f = b_qkv[C:2 * C].copy()
    bv_f = b_qkv[2 * C:3 * C].copy()
    scale = 1.0 / np.sqrt(D)
    for h in range(H):
        sl = slice(h * D, (h + 1) * D)
        wq_f[:, sl] = (wq_f[:, sl] @ rope[h]) * scale
        bq_f[sl] = (bq_f[sl] @ rope[h]) * scale
        wk_f[:, sl] = wk_f[:, sl] @ rope[h]
        bk_f[sl] = bk_f[sl] @ rope[h]

    xT = np.ascontiguousarray(
        np.asarray(x, dtype=np.float32).reshape(NT, C).T)

    in_maps = []
    for m in range(N_CORES):
        sl = slice(m * HB, (m + 1) * HB)
        in_maps.append({
            "xT": xT,
            "wq": np.ascontiguousarray(wq_f[:, sl], dtype=np.float32),
            "wk": np.ascontiguousarray(wk_f[:, sl], dtype=np.float32),
            "wv": np.ascontiguousarray(wv_f[:, sl], dtype=np.float32),
            "bq": np.ascontiguousarray(bq_f[sl, None], dtype=np.float32),
            "bk": np.ascontiguousarray(bk_f[sl, None], dtype=np.float32),
            "bv": np.ascontiguousarray(bv_f[sl, None], dtype=np.float32),
            "wfc": np.ascontiguousarray(w_fc[:, sl], dtype=np.float32),
            "bfc": np.ascontiguousarray(
                np.asarray(b_fc, dtype=np.float32)[sl, None]),
            "ident": np.eye(P, dtype=np.float32),
            "ones": np.ones((P, D), dtype=np.float32),
        })
    return in_maps


def kernel(x, w_qkv, b_qkv, w_fc, b_fc, _trace=False):
    in_maps = _prep_inputs(x, w_qkv, b_qkv, w_fc, b_fc)
    if "nc" not in _CACHE:
        _CACHE["nc"] = build_kernel()
    nc = _CACHE["nc"]
    res = run_bass_kernel_spmd(nc, in_maps, core_ids=list(range(N_CORES)),
                               trace=_trace)
    _CACHE["last_result"] = res
    out = np.concatenate(
        [res.results[m]["outT"].T for m in range(N_CORES)], axis=1)
    return np.ascontiguousarray(out.reshape(B, T, C))


# revision 16
# speedup vs baseline: 1.1635x; 1.0010x over previous
"""Multi-head self-attention (dense transformer block) on 8 Trainium2 cores.

Tensor-parallel over heads: core m handles heads {2m, 2m+1} for both batch
elements. The reference's RoPE uses angles that depend only on the head
index (not the position), so it is a fixed orthogonal rotation per head;
we fold it (and the 1/sqrt(D) score scale) into the QKV weights on the
host. Device pipeline per core:

  1. qT/kT/vT = (w_slice)^T @ x^T          [d-major, tokens on free dim]
  2. V~ = transpose(vT) with a ones column appended per head
  3. per (batch, head): ST = k^T q  -> exp -> PV matmul with V~ gives
     both the output numerator and the softmax denominator (ones col)
  4. normalize via reciprocal + gpsimd partition_broadcast + DVE mult
  5. AllGather the per-core head outputs (hidden^T) per token chunk
  6. column-parallel FC per chunk: out^T slice = w_fc_slice^T @ hidden^T

All matmuls run in float32r (fp32 data, fast PE mode, ~1e-4 rel err).
"""

import numpy as np

import concourse.bass as bass
import concourse.mybir as mybir
from concourse.tile_rust import add_dep_helper
from concourse.bass_utils import run_bass_kernel_spmd
from concourse.tile import TileContext

# Problem shapes (hardcoded per contract)
B, T, C = 2, 2048, 1024
H, D = 16, 64
N_CORES = 8
HPC = H // N_CORES          # heads per core = 2
HB = HPC * D                # head-block width per core = 128
NT = B * T                  # 4096 tokens
P = 128
TCH = 512                   # token chunk (matmul free dim)
F32 = mybir.dt.float32
F32R = mybir.dt.float32r


def _rope_mats():
    """Per-head [D, D] matrices Rt with q_roped_row = q_row @ Rt (row-vector
    convention), matching reference._rope where the angle is head-dependent
    and position-independent."""
    inv_freq = 1.0 / (10000.0 ** (np.arange(0, D, 2, dtype=np.float64) / D))
    mats = []
    for h in range(H):
        theta = h * inv_freq                      # [D/2]
        c, s = np.cos(theta), np.sin(theta)
        R = np.zeros((D, D), dtype=np.float64)
        R[::2, ::2] = np.diag(c)                  # even <- even*cos
        R[1::2, ::2] = -np.diag(s)                # even <- odd*(-sin)
        R[::2, 1::2] = np.diag(s)                 # odd  <- even*sin
        R[1::2, 1::2] = np.diag(c)                # odd  <- odd*cos
        mats.append(R)
    return mats


def split_sync_commands(nc, max_waits=1, max_updates=1):
    """This container's walrus supports only one sync wait / update per
    instruction. Split excess waits into preceding EventSemaphore instrs on
    the same engine queue, and excess updates into following ones."""
    n_split = 0
    for f in nc.m.functions:
        for bb in f.blocks:
            insts = list(bb.instructions)
            new_list = []
            changed = False
            for inst in insts:
                si = inst.sync_info
                waits = list(si.on_wait) if (si and si.on_wait) else []
                if len(waits) > max_waits:
                    for w in waits[max_waits:]:
                        ev = mybir.InstEventSemaphore(
                            name=f"{inst.name}-wsplit-{n_split}",
                            engine=inst.engine, ins=[], outs=[],
                            sync_info=mybir.SyncInfo(on_wait=[w], on_update=[]),
                        )
                        n_split += 1
                        new_list.append(ev)
                    si.on_wait = waits[:max_waits]
                    changed = True
                new_list.append(inst)
                updates = list(si.on_update) if (si and si.on_update) else []
                if len(updates) > max_updates:
                    opcode = type(inst).__name__
                    if "Dma" in opcode or "DMA" in opcode:
                        raise RuntimeError(
                            f"DMA inst {inst.name} has {len(updates)} updates")
                    si.on_update = updates[:max_updates]
                    for u in updates[max_updates:]:
                        ev = mybir.InstEventSemaphore(
                            name=f"{inst.name}-usplit-{n_split}",
                            engine=inst.engine, ins=[], outs=[],
                            sync_info=mybir.SyncInfo(on_wait=[], on_update=[u]),
                        )
                        n_split += 1
                        new_list.append(ev)
                    changed = True
            if changed:
                bb.instructions = new_list
    return n_split


def build_kernel():
    nc = bass.Bass(num_devices=N_CORES)

    xT = nc.dram_tensor("xT", [C, NT], F32R, kind="ExternalInput")
    wq = nc.dram_tensor("wq", [C, HB], F32R, kind="ExternalInput")
    wk = nc.dram_tensor("wk", [C, HB], F32R, kind="ExternalInput")
    wv = nc.dram_tensor("wv", [C, HB], F32R, kind="ExternalInput")
    bq = nc.dram_tensor("bq", [HB, 1], F32, kind="ExternalInput")
    bk = nc.dram_tensor("bk", [HB, 1], F32, kind="ExternalInput")
    bv = nc.dram_tensor("bv", [HB, 1], F32, kind="ExternalInput")
    wfc = nc.dram_tensor("wfc", [C, HB], F32R, kind="ExternalInput")
    bfc = nc.dram_tensor("bfc", [HB, 1], F32, kind="ExternalInput")
    ident_in = nc.dram_tensor("ident", [P, P], F32R, kind="ExternalInput")
    ones_in = nc.dram_tensor("ones", [P, D], F32R, kind="ExternalInput")
    # ones64 for the K=1 denominator-broadcast matmul
    outT = nc.dram_tensor("outT", [HB, NT], F32, kind="ExternalOutput")

    CB = C // P                      # 8 contraction blocks
    SBLK = T // P                    # 16 s-blocks per batch
    NBC = T // TCH                   # 4 chunks per batch
    Exp = mybir.ActivationFunctionType.Exp

    with TileContext(nc) as tc:
        with (
            tc.tile_pool(name="consts", bufs=1) as consts,
            tc.tile_pool(name="qkv", bufs=1) as qkvp,
            tc.tile_pool(name="work", bufs=2) as work,
            tc.tile_pool(name="expp", bufs=4) as expp,
            tc.tile_pool(name="psum", bufs=1, space="PSUM") as psum,
            tc.tile_pool(name="dram", bufs=1, space="DRAM") as dram,
        ):
            # ---- constants ----
            w_sb = {}
            for name, t in (("wq", wq), ("wk", wk), ("wv", wv), ("wfc", wfc)):
                w_t = consts.tile([P, CB, HB], F32R, name=f"{name}_sb")
                nc.sync.dma_start(
                    w_t[:], t[:, :].rearrange("(cb p) o -> p cb o", p=P))
                w_sb[name] = w_t
            b_sb = {}
            for name, t in (("bq", bq), ("bk", bk), ("bv", bv), ("bfc", bfc)):
                b_t = consts.tile([HB, 1], F32, name=f"{name}_sb")
                nc.sync.dma_start(b_t[:], t[:])
                b_sb[name] = b_t
            identity = consts.tile([P, P], F32R, name="identity")
            nc.sync.dma_start(identity[:], ident_in[:])
            ones64 = consts.tile([1, D], F32, name="ones64")
            nc.sync.dma_start(ones64[:], ones_in[0:1, 0:D].bitcast(F32))

            # ---- persistent qkv storage ----
            qT = qkvp.tile([P, NT], F32R, name="qT")
            kz = [qkvp.tile([P, NT], F32R, name=f"kz{h}")
                  for h in range(HPC)]
            vT = qkvp.tile([P, NT], F32R, name="vT")
            # V~: [s-in-block, s-block, 2*(D+1)] with ones at cols D and 2D+1
            vtl = qkvp.tile([P, SBLK * B, 2 * (D + 1)], F32R, name="vtl")
            nc.sync.dma_start(vtl[:, :, D:D + 1],
                              ones_in[:, 0:SBLK * B, None])
            nc.sync.dma_start(vtl[:, :, 2 * D + 1:2 * D + 2],
                              ones_in[:, 0:SBLK * B, None])

            ag_in = {}
            ag_out = {}
            for b in range(B):
                ag_in[b] = dram.tile([HB, T], F32R, name=f"ag_in_{b}",
                                     tag=f"ag_in_{b}")
                ag_out[b] = dram.tile([N_CORES * HB, T], F32R,
                                      name=f"ag_out_{b}", tag=f"ag_out_{b}",
                                      addr_space="Shared")

            def proj_chunk(tcix):
                tsl = slice(tcix * TCH, (tcix + 1) * TCH)
                xt_tiles = []
                for cb in range(CB):
                    xt = work.tile([P, TCH], F32R, tag="xt", name=f"xt_{cb}",
                                   bufs=2 * CB)
                    nc.sync.dma_start(xt[:], xT[cb * P:(cb + 1) * P, tsl])
                    xt_tiles.append(xt)
                for wname, bname, dst in (("wq", "bq", qT), ("wk", "bk", None),
                                          ("wv", "bv", vT)):
                    ps = psum.tile([P, TCH], F32, tag="mm", name="ps_proj",
                                   bufs=3)
                    for cb in range(CB):
                        nc.tensor.matmul(ps[:], w_sb[wname][:, cb, :],
                                         xt_tiles[cb],
                                         start=(cb == 0), stop=(cb == CB - 1))
                    if dst is not None:
                        nc.vector.tensor_scalar_add(dst[:, tsl], ps[:],
                                                    b_sb[bname][:])
                    else:
                        # k: write zero-padded per-head copies for full-K ST
                        nc.vector.tensor_scalar_add(
                            kz[0][0:D, tsl], ps[0:D, :], b_sb["bk"][0:D])
                        nc.vector.tensor_scalar_mul(
                            kz[0][D:P, tsl], ps[D:P, :], 0.0)
                        nc.vector.tensor_scalar_add(
                            kz[1][D:P, tsl], ps[D:P, :], b_sb["bk"][D:P])
                        nc.vector.tensor_scalar_mul(
                            kz[1][0:D, tsl], ps[0:D, :], 0.0)

            def vtl_block(sb):
                pst = psum.tile([P, P], F32R, tag="mmtr", name="ps_tr",
                                bufs=1)
                nc.tensor.transpose(pst[:], vT[:, sb * P:(sb + 1) * P],
                                    identity[:])
                nc.vector.tensor_copy(out=vtl[:, sb, 0:D], in_=pst[:, 0:D])
                nc.vector.tensor_copy(out=vtl[:, sb, D + 1:2 * D + 1],
                                      in_=pst[:, D:2 * D])

            def attention_chunk(b, tcix):
                tsl = slice(b * T + tcix * TCH, b * T + (tcix + 1) * TCH)
                pv_ps = [
                    psum.tile([P, TCH], F32, tag=f"pv{h}",
                              name=f"ps_pv{h}", bufs=2)
                    for h in range(HPC)
                ]
                for sb in range(SBLK):
                    gsb = b * SBLK + sb
                    ssl = slice(b * T + sb * P, b * T + sb * P + P)
                    e_tiles = []
                    for h in range(HPC):
                        ps_st = psum.tile([P, TCH], F32, tag="mm",
                                          name="ps_st", bufs=3)
                        nc.tensor.matmul(ps_st[:], kz[h][:, ssl],
                                         qT[:, tsl], start=True, stop=True)
                        e = expp.tile([P, TCH], F32R, tag=f"e{h}",
                                      name=f"e{h}", bufs=4)
                        nc.scalar.activation(e[:], ps_st[:], Exp)
                        e_tiles.append(e)
                    for h in range(HPC):
                        last_pv = nc.tensor.matmul(
                            pv_ps[h][0:D + 1, :],
                            vtl[:, gsb, h * (D + 1):(h + 1) * (D + 1)],
                            e_tiles[h][:],
                            start=(sb == 0), stop=(sb == SBLK - 1))
                return pv_ps, last_pv

            def normalize_chunk(b, tcix, pv_ps):
                otsl = slice(tcix * TCH, (tcix + 1) * TCH)
                for h in range(HPC):
                    recip = work.tile([1, TCH], F32, tag="recip",
                                      name="recip", bufs=2)
                    nc.vector.reciprocal(recip[:], pv_ps[h][D:D + 1, :])
                    ps_bc = psum.tile([D, TCH], F32, tag="mmtr",
                                      name="ps_bc", bufs=1)
                    nc.tensor.matmul(ps_bc[:], ones64[:], recip[:],
                                     start=True, stop=True)
                    bc_sb = work.tile([D, TCH], F32R, tag="bc",
                                      name="bc_sb", bufs=2)
                    nc.vector.tensor_copy(out=bc_sb[:], in_=ps_bc[:])
                    ot = work.tile([D, TCH], F32R, tag="ot", name="ot",
                                   bufs=2)
                    nc.vector.tensor_mul(out=ot[:], in0=pv_ps[h][0:D, :],
                                         in1=bc_sb[:])
                    nc.sync.dma_start(
                        ag_in[b][h * D:(h + 1) * D, otsl], ot[:])

            def allgather(b):
                nc.gpsimd.collective_compute(
                    "AllGather", mybir.AluOpType.bypass,
                    replica_groups=[list(range(N_CORES))],
                    ins=[ag_in[b][:].opt()],
                    outs=[ag_out[b][:].opt()])

            def fc_chunk(b, tcix, after=None):
                tsl = slice(b * T + tcix * TCH, b * T + (tcix + 1) * TCH)
                otsl = slice(tcix * TCH, (tcix + 1) * TCH)
                z_tiles = []
                for cb in range(CB):
                    z = work.tile([P, TCH], F32R, tag="z", name=f"z_{cb}",
                                  bufs=2 * CB)
                    nc.gpsimd.dma_start(
                        z[:], ag_out[b][cb * P:(cb + 1) * P, otsl])
                    z_tiles.append(z)
                ps = psum.tile([P, TCH], F32, tag="mm", name="ps_fc",
                               bufs=3)
                for cb in range(CB):
                    mm = nc.tensor.matmul(ps[:], w_sb["wfc"][:, cb, :],
                                          z_tiles[cb],
                                          start=(cb == 0),
                                          stop=(cb == CB - 1))
                    if cb == 0 and after is not None:
                        add_dep_helper(mm.ins, after.ins, sync=False,
                                       reason="fc after attention chunk")
                osb = work.tile([P, TCH], F32, tag="osb", name="osb", bufs=2)
                nc.vector.tensor_scalar_add(osb[:], ps[:], b_sb["bfc"][:])
                nc.gpsimd.dma_start(outT[:, tsl], osb[:])

            # emission order == scheduler priority. proj/FC matmuls act as
            # PE gap-fillers during the exp-bound attention phases.
            for tcix in range(NBC):
                proj_chunk(tcix)
            for sb in range(SBLK):
                vtl_block(sb)
            pending = None
            for tcix in range(NBC):
                pv0, _ = attention_chunk(0, tcix)
                if pending is not None:
                    normalize_chunk(0, pending[0], pending[1])
                pending = (tcix, pv0)
            normalize_chunk(0, pending[0], pending[1])
            allgather(0)
            for tcix in range(NBC, 2 * NBC):
                proj_chunk(tcix)
            for sb in range(SBLK, 2 * SBLK):
                vtl_block(sb)
            pending = None
            for tcix in range(NBC):
                pv1, last_pv = attention_chunk(1, tcix)
                if pending is not None:
                    normalize_chunk(1, pending[0], pending[1])
                pending = (tcix, pv1)
                fc_chunk(0, tcix, after=last_pv)
            normalize_chunk(1, pending[0], pending[1])
            allgather(1)
            for tcix in range(NBC):
                fc_chunk(1, tcix)

    split_sync_commands(nc)
    return nc


_CACHE = {}


def _prep_inputs(x, w_qkv, b_qkv, w_fc, b_fc):
    """Host-side: fold RoPE + scale into weights, shard per core."""
    rope = _rope_mats()
    w_qkv = np.asarray(w_qkv, dtype=np.float64)
    b_qkv = np.asarray(b_qkv, dtype=np.float64)
    wq_f = w_qkv[:, 0:C].copy()
    wk_f = w_qkv[:, C:2 * C].copy()
    wv_f = w_qkv[:, 2 * C:3 * C].copy()
    bq_f = b_qkv[0:C].copy()
    bk_f = b_qkv[C:2 * C].copy()
    bv_f = b_qkv[2 * C:3 * C].copy()
    scale = 1.0 / np.sqrt(D)
    for h in range(H):
        sl = slice(h * D, (h + 1) * D)
        wq_f[:, sl] = (wq_f[:, sl] @ rope[h]) * scale
        bq_f[sl] = (bq_f[sl] @ rope[h]) * scale
        wk_f[:, sl] = wk_f[:, sl] @ rope[h]
        bk_f[sl] = bk_f[sl] @ rope[h]

    xT = np.ascontiguousarray(
        np.asarray(x, dtype=np.float32).reshape(NT, C).T)

    in_maps = []
    for m in range(N_CORES):
        sl = slice(m * HB, (m + 1) * HB)
        in_maps.append({
            "xT": xT,
            "wq": np.ascontiguousarray(wq_f[:, sl], dtype=np.float32),
            "wk": np.ascontiguousarray(wk_f[:, sl], dtype=np.float32),
            "wv": np.ascontiguousarray(wv_f[:, sl], dtype=np.float32),
            "bq": np.ascontiguousarray(bq_f[sl, None], dtype=np.float32),
            "bk": np.ascontiguousarray(bk_f[sl, None], dtype=np.float32),
            "bv": np.ascontiguousarray(bv_f[sl, None], dtype=np.float32),
            "wfc": np.ascontiguousarray(w_fc[:, sl], dtype=np.float32),
            "bfc": np.ascontiguousarray(
                np.asarray(b_fc, dtype=np.float32)[sl, None]),
            "ident": np.eye(P, dtype=np.float32),
            "ones": np.ones((P, D), dtype=np.float32),
        })
    return in_maps


def kernel(x, w_qkv, b_qkv, w_fc, b_fc, _trace=False):
    in_maps = _prep_inputs(x, w_qkv, b_qkv, w_fc, b_fc)
    if "nc" not in _CACHE:
        _CACHE["nc"] = build_kernel()
    nc = _CACHE["nc"]
    res = run_bass_kernel_spmd(nc, in_maps, core_ids=list(range(N_CORES)),
                               trace=_trace)
    _CACHE["last_result"] = res
    out = np.concatenate(
        [res.results[m]["outT"].T for m in range(N_CORES)], axis=1)
    return np.ascontiguousarray(out.reshape(B, T, C))


# revision 17
# speedup vs baseline: 1.3031x; 1.1200x over previous
"""Multi-head self-attention (dense transformer block) on 8 Trainium2 cores.

Tensor-parallel over heads: core m handles heads {2m, 2m+1} for both batch
elements. The reference's RoPE uses angles that depend only on the head
index (not the position), so it is a fixed orthogonal rotation per head;
we fold it (and the 1/sqrt(D) score scale) into the QKV weights on the
host. Device pipeline per core:

  1. qT/kT/vT = (w_slice)^T @ x^T          [d-major, tokens on free dim]
  2. V~ = transpose(vT) with a ones column appended per head
  3. per (batch, head): ST = k^T q  -> exp -> PV matmul with V~ gives
     both the output numerator and the softmax denominator (ones col)
  4. normalize via reciprocal + gpsimd partition_broadcast + DVE mult
  5. AllGather the per-core head outputs (hidden^T) per token chunk
  6. column-parallel FC per chunk: out^T slice = w_fc_slice^T @ hidden^T

All matmuls run in float32r (fp32 data, fast PE mode, ~1e-4 rel err).
"""

import numpy as np

import concourse.bass as bass
import concourse.mybir as mybir
from concourse.tile_rust import add_dep_helper
from concourse.bass_utils import run_bass_kernel_spmd
from concourse.tile import TileContext

# Problem shapes (hardcoded per contract)
B, T, C = 2, 2048, 1024
H, D = 16, 64
N_CORES = 8
HPC = H // N_CORES          # heads per core = 2
HB = HPC * D                # head-block width per core = 128
NT = B * T                  # 4096 tokens
P = 128
TCH = 512                   # token chunk (matmul free dim)
F32 = mybir.dt.float32
F32R = mybir.dt.float32r


def _rope_mats():
    """Per-head [D, D] matrices Rt with q_roped_row = q_row @ Rt (row-vector
    convention), matching reference._rope where the angle is head-dependent
    and position-independent."""
    inv_freq = 1.0 / (10000.0 ** (np.arange(0, D, 2, dtype=np.float64) / D))
    mats = []
    for h in range(H):
        theta = h * inv_freq                      # [D/2]
        c, s = np.cos(theta), np.sin(theta)
        R = np.zeros((D, D), dtype=np.float64)
        R[::2, ::2] = np.diag(c)                  # even <- even*cos
        R[1::2, ::2] = -np.diag(s)                # even <- odd*(-sin)
        R[::2, 1::2] = np.diag(s)                 # odd  <- even*sin
        R[1::2, 1::2] = np.diag(c)                # odd  <- odd*cos
        mats.append(R)
    return mats


def split_sync_commands(nc, max_waits=1, max_updates=1):
    """This container's walrus supports only one sync wait / update per
    instruction. Split excess waits into preceding EventSemaphore instrs on
    the same engine queue, and excess updates into following ones."""
    n_split = 0
    for f in nc.m.functions:
        for bb in f.blocks:
            insts = list(bb.instructions)
            new_list = []
            changed = False
            for inst in insts:
                si = inst.sync_info
                waits = list(si.on_wait) if (si and si.on_wait) else []
                if len(waits) > max_waits:
                    for w in waits[max_waits:]:
                        ev = mybir.InstEventSemaphore(
                            name=f"{inst.name}-wsplit-{n_split}",
                            engine=inst.engine, ins=[], outs=[],
                            sync_info=mybir.SyncInfo(on_wait=[w], on_update=[]),
                        )
                        n_split += 1
                        new_list.append(ev)
                    si.on_wait = waits[:max_waits]
                    changed = True
                new_list.append(inst)
                updates = list(si.on_update) if (si and si.on_update) else []
                if len(updates) > max_updates:
                    opcode = type(inst).__name__
                    if "Dma" in opcode or "DMA" in opcode:
                        raise RuntimeError(
                            f"DMA inst {inst.name} has {len(updates)} updates")
                    si.on_update = updates[:max_updates]
                    for u in updates[max_updates:]:
                        ev = mybir.InstEventSemaphore(
                            name=f"{inst.name}-usplit-{n_split}",
                            engine=inst.engine, ins=[], outs=[],
                            sync_info=mybir.SyncInfo(on_wait=[], on_update=[u]),
                        )
                        n_split += 1
                        new_list.append(ev)
                    changed = True
            if changed:
                bb.instructions = new_list
    return n_split


def build_kernel():
    nc = bass.Bass(num_devices=N_CORES)

    xT = nc.dram_tensor("xT", [C, NT], F32R, kind="ExternalInput")
    wq = nc.dram_tensor("wq", [C, HB], F32R, kind="ExternalInput")
    wk = nc.dram_tensor("wk", [C, HB], F32R, kind="ExternalInput")
    wv = nc.dram_tensor("wv", [C, HB], F32R, kind="ExternalInput")
    bq = nc.dram_tensor("bq", [HB, 1], F32, kind="ExternalInput")
    bk = nc.dram_tensor("bk", [HB, 1], F32, kind="ExternalInput")
    bv = nc.dram_tensor("bv", [HB, 1], F32, kind="ExternalInput")
    wfc = nc.dram_tensor("wfc", [C, HB], F32R, kind="ExternalInput")
    bfc = nc.dram_tensor("bfc", [HB, 1], F32, kind="ExternalInput")
    ident_in = nc.dram_tensor("ident", [P, P], F32R, kind="ExternalInput")
    ones_in = nc.dram_tensor("ones", [P, D], F32R, kind="ExternalInput")
    # ones64 for the K=1 denominator-broadcast matmul
    outT = nc.dram_tensor("outT", [HB, NT], F32, kind="ExternalOutput")

    CB = C // P                      # 8 contraction blocks
    SBLK = T // P                    # 16 s-blocks per batch
    NBC = T // TCH                   # 4 chunks per batch
    Exp = mybir.ActivationFunctionType.Exp

    with TileContext(nc) as tc:
        with (
            tc.tile_pool(name="consts", bufs=1) as consts,
            tc.tile_pool(name="qkv", bufs=1) as qkvp,
            tc.tile_pool(name="work", bufs=2) as work,
            tc.tile_pool(name="expp", bufs=4) as expp,
            tc.tile_pool(name="psum", bufs=1, space="PSUM") as psum,
            tc.tile_pool(name="dram", bufs=1, space="DRAM") as dram,
        ):
            # ---- constants ----
            w_sb = {}
            for name, t in (("wq", wq), ("wk", wk), ("wv", wv), ("wfc", wfc)):
                w_t = consts.tile([P, CB, HB], F32R, name=f"{name}_sb")
                nc.sync.dma_start(
                    w_t[:], t[:, :].rearrange("(cb p) o -> p cb o", p=P))
                w_sb[name] = w_t
            b_sb = {}
            for name, t in (("bq", bq), ("bk", bk), ("bv", bv), ("bfc", bfc)):
                b_t = consts.tile([HB, 1], F32, name=f"{name}_sb")
                nc.sync.dma_start(b_t[:], t[:])
                b_sb[name] = b_t
            identity = consts.tile([P, P], F32R, name="identity")
            nc.sync.dma_start(identity[:], ident_in[:])
            ones64 = consts.tile([1, D], F32, name="ones64")
            nc.sync.dma_start(ones64[:], ones_in[0:1, 0:D].bitcast(F32))

            # ---- persistent qkv storage ----
            qT = qkvp.tile([P, NT], F32R, name="qT")
            kz = [qkvp.tile([P, NT], F32R, name=f"kz{h}")
                  for h in range(HPC)]
            vT = qkvp.tile([P, NT], F32R, name="vT")
            # V~: [s-in-block, s-block, 2*(D+1)] with ones at cols D and 2D+1
            vtl = qkvp.tile([P, SBLK * B, 2 * (D + 1)], F32R, name="vtl")
            nc.sync.dma_start(vtl[:, :, D:D + 1],
                              ones_in[:, 0:SBLK * B, None])
            nc.sync.dma_start(vtl[:, :, 2 * D + 1:2 * D + 2],
                              ones_in[:, 0:SBLK * B, None])

            ag_in = {}
            ag_out = {}
            ag_in[0] = dram.tile([HB, T], F32R, name="ag_in_0", tag="ag_in_0")
            ag_out[0] = dram.tile([N_CORES * HB, T], F32R, name="ag_out_0",
                                  tag="ag_out_0", addr_space="Shared")
            ag_in["1a"] = dram.tile([HB, 3 * TCH], F32R, name="ag_in_1a",
                                    tag="ag_in_1a")
            ag_out["1a"] = dram.tile([N_CORES * HB, 3 * TCH], F32R,
                                     name="ag_out_1a", tag="ag_out_1a",
                                     addr_space="Shared")
            ag_in["1b"] = dram.tile([HB, TCH], F32R, name="ag_in_1b",
                                    tag="ag_in_1b")
            ag_out["1b"] = dram.tile([N_CORES * HB, TCH], F32R,
                                     name="ag_out_1b", tag="ag_out_1b",
                                     addr_space="Shared")

            def proj_chunk(tcix):
                tsl = slice(tcix * TCH, (tcix + 1) * TCH)
                xt_tiles = []
                for cb in range(CB):
                    xt = work.tile([P, TCH], F32R, tag="xt", name=f"xt_{cb}",
                                   bufs=2 * CB)
                    nc.sync.dma_start(xt[:], xT[cb * P:(cb + 1) * P, tsl])
                    xt_tiles.append(xt)
                for wname, bname, dst in (("wq", "bq", qT), ("wk", "bk", None),
                                          ("wv", "bv", vT)):
                    ps = psum.tile([P, TCH], F32, tag="mm", name="ps_proj",
                                   bufs=3)
                    for cb in range(CB):
                        nc.tensor.matmul(ps[:], w_sb[wname][:, cb, :],
                                         xt_tiles[cb],
                                         start=(cb == 0), stop=(cb == CB - 1))
                    if dst is not None:
                        nc.vector.tensor_scalar_add(dst[:, tsl], ps[:],
                                                    b_sb[bname][:])
                    else:
                        # k: write zero-padded per-head copies for full-K ST
                        nc.vector.tensor_scalar_add(
                            kz[0][0:D, tsl], ps[0:D, :], b_sb["bk"][0:D])
                        nc.vector.tensor_scalar_mul(
                            kz[0][D:P, tsl], ps[D:P, :], 0.0)
                        nc.vector.tensor_scalar_add(
                            kz[1][D:P, tsl], ps[D:P, :], b_sb["bk"][D:P])
                        nc.vector.tensor_scalar_mul(
                            kz[1][0:D, tsl], ps[0:D, :], 0.0)

            def vtl_block(sb):
                pst = psum.tile([P, P], F32R, tag="mmtr", name="ps_tr",
                                bufs=1)
                nc.tensor.transpose(pst[:], vT[:, sb * P:(sb + 1) * P],
                                    identity[:])
                nc.vector.tensor_copy(out=vtl[:, sb, 0:D], in_=pst[:, 0:D])
                nc.vector.tensor_copy(out=vtl[:, sb, D + 1:2 * D + 1],
                                      in_=pst[:, D:2 * D])

            def attention_chunk(b, tcix):
                tsl = slice(b * T + tcix * TCH, b * T + (tcix + 1) * TCH)
                pv_ps = [
                    psum.tile([P, TCH], F32, tag=f"pv{h}",
                              name=f"ps_pv{h}", bufs=2)
                    for h in range(HPC)
                ]
                for sb in range(SBLK):
                    gsb = b * SBLK + sb
                    ssl = slice(b * T + sb * P, b * T + sb * P + P)
                    e_tiles = []
                    for h in range(HPC):
                        ps_st = psum.tile([P, TCH], F32, tag="mm",
                                          name="ps_st", bufs=3)
                        nc.tensor.matmul(ps_st[:], kz[h][:, ssl],
                                         qT[:, tsl], start=True, stop=True)
                        e = expp.tile([P, TCH], F32R, tag=f"e{h}",
                                      name=f"e{h}", bufs=4)
                        nc.scalar.activation(e[:], ps_st[:], Exp)
                        e_tiles.append(e)
                    for h in range(HPC):
                        last_pv = nc.tensor.matmul(
                            pv_ps[h][0:D + 1, :],
                            vtl[:, gsb, h * (D + 1):(h + 1) * (D + 1)],
                            e_tiles[h][:],
                            start=(sb == 0), stop=(sb == SBLK - 1))
                return pv_ps, last_pv

            def normalize_chunk(b, tcix, pv_ps):
                if b == 0:
                    dst, otsl = ag_in[0], slice(tcix * TCH, (tcix + 1) * TCH)
                elif tcix < 3:
                    dst, otsl = ag_in["1a"], slice(tcix * TCH,
                                                   (tcix + 1) * TCH)
                else:
                    dst, otsl = ag_in["1b"], slice(0, TCH)
                for h in range(HPC):
                    recip = work.tile([1, TCH], F32, tag="recip",
                                      name="recip", bufs=2)
                    nc.vector.reciprocal(recip[:], pv_ps[h][D:D + 1, :])
                    rb = dram.tile([1, TCH], F32, tag="recip_bounce",
                                   name="rb", bufs=2)
                    nc.sync.dma_start(rb[:], recip[:])
                    bc_sb = work.tile([D, TCH], F32, tag="bc",
                                      name="bc_sb", bufs=2)
                    nc.sync.dma_start(bc_sb[:],
                                      rb[0:1, :].to_broadcast([D, TCH]))
                    ot = work.tile([D, TCH], F32R, tag="ot", name="ot",
                                   bufs=2)
                    nc.vector.tensor_mul(out=ot[:], in0=pv_ps[h][0:D, :],
                                         in1=bc_sb[:])
                    nc.sync.dma_start(dst[h * D:(h + 1) * D, otsl], ot[:])

            def allgather(key):
                nc.gpsimd.collective_compute(
                    "AllGather", mybir.AluOpType.bypass,
                    replica_groups=[list(range(N_CORES))],
                    ins=[ag_in[key][:].opt()],
                    outs=[ag_out[key][:].opt()])

            def fc_chunk(b, tcix, after=None):
                tsl = slice(b * T + tcix * TCH, b * T + (tcix + 1) * TCH)
                if b == 0:
                    zsrc, otsl = ag_out[0], slice(tcix * TCH,
                                                  (tcix + 1) * TCH)
                elif tcix < 3:
                    zsrc, otsl = ag_out["1a"], slice(tcix * TCH,
                                                     (tcix + 1) * TCH)
                else:
                    zsrc, otsl = ag_out["1b"], slice(0, TCH)
                z_tiles = []
                for cb in range(CB):
                    z = work.tile([P, TCH], F32R, tag="z", name=f"z_{cb}",
                                  bufs=2 * CB)
                    if b == 0:
                        nc.gpsimd.dma_start(
                            z[:], zsrc[cb * P:(cb + 1) * P, otsl])
                    else:
                        nc.sync.dma_start(
                            z[:], zsrc[cb * P:(cb + 1) * P, otsl])
                    z_tiles.append(z)
                ps = psum.tile([P, TCH], F32, tag="mm", name="ps_fc",
                               bufs=3)
                for cb in range(CB):
                    mm = nc.tensor.matmul(ps[:], w_sb["wfc"][:, cb, :],
                                          z_tiles[cb],
                                          start=(cb == 0),
                                          stop=(cb == CB - 1))
                    if cb == 0 and after is not None:
                        add_dep_helper(mm.ins, after.ins, sync=False,
                                       reason="fc after attention chunk")
                osb = work.tile([P, TCH], F32, tag="osb", name="osb", bufs=2)
                nc.vector.tensor_scalar_add(osb[:], ps[:], b_sb["bfc"][:])
                if b == 0:
                    nc.gpsimd.dma_start(outT[:, tsl], osb[:])
                else:
                    nc.sync.dma_start(outT[:, tsl], osb[:])

            # emission order == scheduler priority. proj/FC matmuls act as
            # PE gap-fillers during the exp-bound attention phases.
            for tcix in range(NBC):
                proj_chunk(tcix)
            for sb in range(SBLK):
                vtl_block(sb)
            pending = None
            for tcix in range(NBC):
                pv0, _ = attention_chunk(0, tcix)
                if pending is not None:
                    normalize_chunk(0, pending[0], pending[1])
                pending = (tcix, pv0)
            normalize_chunk(0, pending[0], pending[1])
            allgather(0)
            for tcix in range(NBC, 2 * NBC):
                proj_chunk(tcix)
            for sb in range(SBLK, 2 * SBLK):
                vtl_block(sb)
            pending = None
            for tcix in range(NBC):
                pv1, last_pv = attention_chunk(1, tcix)
                if pending is not None:
                    normalize_chunk(1, pending[0], pending[1])
                    if pending[0] == 2:
                        allgather("1a")
                pending = (tcix, pv1)
                fc_chunk(0, tcix, after=last_pv)
            normalize_chunk(1, pending[0], pending[1])
            allgather("1b")
            for tcix in range(NBC):
                fc_chunk(1, tcix)

    split_sync_commands(nc)
    return nc


_CACHE = {}


def _prep_inputs(x, w_qkv, b_qkv, w_fc, b_fc):
    """Host-side: fold RoPE + scale into weights, shard per core."""
    rope = _rope_mats()
    w_qkv = np.asarray(w_qkv, dtype=np.float64)
    b_qkv = np.asarray(b_qkv, dtype=np.float64)
    wq_f = w_qkv[:, 0:C].copy()
    wk_f = w_qkv[:, C:2 * C].copy()
    wv_f = w_qkv[:, 2 * C:3 * C].copy()
    bq_f = b_qkv[0:C].copy()
    bk_f = b_qkv[C:2 * C].copy()
    bv_f = b_qkv[2 * C:3 * C].copy()
    scale = 1.0 / np.sqrt(D)
    for h in range(H):
        sl = slice(h * D, (h + 1) * D)
        wq_f[:, sl] = (wq_f[:, sl] @ rope[h]) * scale
        bq_f[sl] = (bq_f[sl] @ rope[h]) * scale
        wk_f[:, sl] = wk_f[:, sl] @ rope[h]
        bk_f[sl] = bk_f[sl] @ rope[h]

    xT = np.ascontiguousarray(
        np.asarray(x, dtype=np.float32).reshape(NT, C).T)

    in_maps = []
    for m in range(N_CORES):
        sl = slice(m * HB, (m + 1) * HB)
        in_maps.append({
            "xT": xT,
            "wq": np.ascontiguousarray(wq_f[:, sl], dtype=np.float32),
            "wk": np.ascontiguousarray(wk_f[:, sl], dtype=np.float32),
            "wv": np.ascontiguousarray(wv_f[:, sl], dtype=np.float32),
            "bq": np.ascontiguousarray(bq_f[sl, None], dtype=np.float32),
            "bk": np.ascontiguousarray(bk_f[sl, None], dtype=np.float32),
            "bv": np.ascontiguousarray(bv_f[sl, None], dtype=np.float32),
            "wfc": np.ascontiguousarray(w_fc[:, sl], dtype=np.float32),
            "bfc": np.ascontiguousarray(
                np.asarray(b_fc, dtype=np.float32)[sl, None]),
            "ident": np.eye(P, dtype=np.float32),
            "ones": np.ones((P, D), dtype=np.float32),
        })
    return in_maps


def kernel(x, w_qkv, b_qkv, w_fc, b_fc, _trace=False):
    in_maps = _prep_inputs(x, w_qkv, b_qkv, w_fc, b_fc)
    if "nc" not in _CACHE:
        _CACHE["nc"] = build_kernel()
    nc = _CACHE["nc"]
    res = run_bass_kernel_spmd(nc, in_maps, core_ids=list(range(N_CORES)),
                               trace=_trace)
    _CACHE["last_result"] = res
    out = np.concatenate(
        [res.results[m]["outT"].T for m in range(N_CORES)], axis=1)
    return np.ascontiguousarray(out.reshape(B, T, C))


# revision 18
# speedup vs baseline: 1.3607x; 1.0442x over previous
"""Multi-head self-attention (dense transformer block) on 8 Trainium2 cores.

Tensor-parallel over heads: core m handles heads {2m, 2m+1} for both batch
elements. The reference's RoPE uses angles that depend only on the head
index (not the position), so it is a fixed orthogonal rotation per head;
we fold it (and the 1/sqrt(D) score scale) into the QKV weights on the
host. Device pipeline per core:

  1. qT/kT/vT = (w_slice)^T @ x^T          [d-major, tokens on free dim]
  2. V~ = transpose(vT) with a ones column appended per head
  3. per (batch, head): ST = k^T q  -> exp -> PV matmul with V~ gives
     both the output numerator and the softmax denominator (ones col)
  4. normalize via reciprocal + gpsimd partition_broadcast + DVE mult
  5. AllGather the per-core head outputs (hidden^T) per token chunk
  6. column-parallel FC per chunk: out^T slice = w_fc_slice^T @ hidden^T

All matmuls run in float32r (fp32 data, fast PE mode, ~1e-4 rel err).
"""

import numpy as np

import concourse.bass as bass
import concourse.mybir as mybir
from concourse.tile_rust import add_dep_helper
from concourse.bass_utils import run_bass_kernel_spmd
from concourse.tile import TileContext

# Problem shapes (hardcoded per contract)
B, T, C = 2, 2048, 1024
H, D = 16, 64
N_CORES = 8
HPC = H // N_CORES          # heads per core = 2
HB = HPC * D                # head-block width per core = 128
NT = B * T                  # 4096 tokens
P = 128
TCH = 512                   # token chunk (matmul free dim)
F32 = mybir.dt.float32
F32R = mybir.dt.float32r


def _rope_mats():
    """Per-head [D, D] matrices Rt with q_roped_row = q_row @ Rt (row-vector
    convention), matching reference._rope where the angle is head-dependent
    and position-independent."""
    inv_freq = 1.0 / (10000.0 ** (np.arange(0, D, 2, dtype=np.float64) / D))
    mats = []
    for h in range(H):
        theta = h * inv_freq                      # [D/2]
        c, s = np.cos(theta), np.sin(theta)
        R = np.zeros((D, D), dtype=np.float64)
        R[::2, ::2] = np.diag(c)                  # even <- even*cos
        R[1::2, ::2] = -np.diag(s)                # even <- odd*(-sin)
        R[::2, 1::2] = np.diag(s)                 # odd  <- even*sin
        R[1::2, 1::2] = np.diag(c)                # odd  <- odd*cos
        mats.append(R)
    return mats


def split_sync_commands(nc, max_waits=1, max_updates=1):
    """This container's walrus supports only one sync wait / update per
    instruction. Split excess waits into preceding EventSemaphore instrs on
    the same engine queue, and excess updates into following ones."""
    n_split = 0
    for f in nc.m.functions:
        for bb in f.blocks:
            insts = list(bb.instructions)
            new_list = []
            changed = False
            for inst in insts:
                si = inst.sync_info
                waits = list(si.on_wait) if (si and si.on_wait) else []
                if len(waits) > max_waits:
                    for w in waits[max_waits:]:
                        ev = mybir.InstEventSemaphore(
                            name=f"{inst.name}-wsplit-{n_split}",
                            engine=inst.engine, ins=[], outs=[],
                            sync_info=mybir.SyncInfo(on_wait=[w], on_update=[]),
                        )
                        n_split += 1
                        new_list.append(ev)
                    si.on_wait = waits[:max_waits]
                    changed = True
                new_list.append(inst)
                updates = list(si.on_update) if (si and si.on_update) else []
                if len(updates) > max_updates:
                    opcode = type(inst).__name__
                    if "Dma" in opcode or "DMA" in opcode:
                        raise RuntimeError(
                            f"DMA inst {inst.name} has {len(updates)} updates")
                    si.on_update = updates[:max_updates]
                    for u in updates[max_updates:]:
                        ev = mybir.InstEventSemaphore(
                            name=f"{inst.name}-usplit-{n_split}",
                            engine=inst.engine, ins=[], outs=[],
                            sync_info=mybir.SyncInfo(on_wait=[], on_update=[u]),
                        )
                        n_split += 1
                        new_list.append(ev)
                    changed = True
            if changed:
                bb.instructions = new_list
    return n_split


def build_kernel():
    nc = bass.Bass(num_devices=N_CORES)

    xT = nc.dram_tensor("xT", [C, NT], F32R, kind="ExternalInput")
    wq = nc.dram_tensor("wq", [C, HB], F32R, kind="ExternalInput")
    wk = nc.dram_tensor("wk", [C, HB], F32R, kind="ExternalInput")
    wv = nc.dram_tensor("wv", [C, HB], F32R, kind="ExternalInput")
    bq = nc.dram_tensor("bq", [HB, 1], F32, kind="ExternalInput")
    bk = nc.dram_tensor("bk", [HB, 1], F32, kind="ExternalInput")
    bv = nc.dram_tensor("bv", [HB, 1], F32, kind="ExternalInput")
    wfc = nc.dram_tensor("wfc", [C, HB], F32R, kind="ExternalInput")
    bfc = nc.dram_tensor("bfc", [HB, 1], F32, kind="ExternalInput")
    ident_in = nc.dram_tensor("ident", [P, P], F32R, kind="ExternalInput")
    ones_in = nc.dram_tensor("ones", [P, D], F32R, kind="ExternalInput")
    # ones64 for the K=1 denominator-broadcast matmul
    outT = nc.dram_tensor("outT", [HB, NT], F32, kind="ExternalOutput")

    CB = C // P                      # 8 contraction blocks
    SBLK = T // P                    # 16 s-blocks per batch
    NBC = T // TCH                   # 4 chunks per batch
    Exp = mybir.ActivationFunctionType.Exp

    with TileContext(nc) as tc:
        with (
            tc.tile_pool(name="consts", bufs=1) as consts,
            tc.tile_pool(name="qkv", bufs=1) as qkvp,
            tc.tile_pool(name="work", bufs=2) as work,
            tc.tile_pool(name="expp", bufs=4) as expp,
            tc.tile_pool(name="psum", bufs=1, space="PSUM") as psum,
            tc.tile_pool(name="dram", bufs=1, space="DRAM") as dram,
        ):
            # ---- constants ----
            w_sb = {}
            for name, t in (("wq", wq), ("wk", wk), ("wv", wv), ("wfc", wfc)):
                w_t = consts.tile([P, CB, HB], F32R, name=f"{name}_sb")
                nc.sync.dma_start(
                    w_t[:], t[:, :].rearrange("(cb p) o -> p cb o", p=P))
                w_sb[name] = w_t
            b_sb = {}
            for name, t in (("bq", bq), ("bk", bk), ("bv", bv), ("bfc", bfc)):
                b_t = consts.tile([HB, 1], F32, name=f"{name}_sb")
                nc.sync.dma_start(b_t[:], t[:])
                b_sb[name] = b_t
            identity = consts.tile([P, P], F32R, name="identity")
            nc.sync.dma_start(identity[:], ident_in[:])
            ones64 = consts.tile([1, D], F32, name="ones64")
            nc.sync.dma_start(ones64[:], ones_in[0:1, 0:D].bitcast(F32))

            # ---- persistent qkv storage ----
            qT = qkvp.tile([P, NT], F32R, name="qT")
            kz = [qkvp.tile([P, NT], F32R, name=f"kz{h}")
                  for h in range(HPC)]
            vT = qkvp.tile([P, NT], F32R, name="vT")
            # V~: [s-in-block, s-block, 2*(D+1)] with ones at cols D and 2D+1
            vtl = qkvp.tile([P, SBLK * B, 2 * (D + 1)], F32R, name="vtl")
            nc.sync.dma_start(vtl[:, :, D:D + 1],
                              ones_in[:, 0:SBLK * B, None])
            nc.sync.dma_start(vtl[:, :, 2 * D + 1:2 * D + 2],
                              ones_in[:, 0:SBLK * B, None])

            ag_in = {}
            ag_out = {}
            ag_in[0] = dram.tile([HB, T], F32R, name="ag_in_0", tag="ag_in_0")
            ag_out[0] = dram.tile([N_CORES * HB, T], F32R, name="ag_out_0",
                                  tag="ag_out_0", addr_space="Shared")
            ag_in["1a"] = dram.tile([HB, 3 * TCH], F32R, name="ag_in_1a",
                                    tag="ag_in_1a")
            ag_out["1a"] = dram.tile([N_CORES * HB, 3 * TCH], F32R,
                                     name="ag_out_1a", tag="ag_out_1a",
                                     addr_space="Shared")
            ag_in["1b"] = dram.tile([HB, TCH], F32R, name="ag_in_1b",
                                    tag="ag_in_1b")
            ag_out["1b"] = dram.tile([N_CORES * HB, TCH], F32R,
                                     name="ag_out_1b", tag="ag_out_1b",
                                     addr_space="Shared")

            def proj_chunk(tcix):
                tsl = slice(tcix * TCH, (tcix + 1) * TCH)
                xt_tiles = []
                for cb in range(CB):
                    xt = work.tile([P, TCH], F32R, tag="xt", name=f"xt_{cb}",
                                   bufs=2 * CB)
                    nc.sync.dma_start(xt[:], xT[cb * P:(cb + 1) * P, tsl])
                    xt_tiles.append(xt)
                for wname, bname, dst in (("wq", "bq", qT), ("wk", "bk", None),
                                          ("wv", "bv", vT)):
                    ps = psum.tile([P, TCH], F32, tag="mm", name="ps_proj",
                                   bufs=3)
                    for cb in range(CB):
                        nc.tensor.matmul(ps[:], w_sb[wname][:, cb, :],
                                         xt_tiles[cb],
                                         start=(cb == 0), stop=(cb == CB - 1))
                    if dst is not None:
                        nc.vector.tensor_scalar_add(dst[:, tsl], ps[:],
                                                    b_sb[bname][:])
                    else:
                        # k: write zero-padded per-head copies for full-K ST
                        nc.vector.tensor_scalar_add(
                            kz[0][0:D, tsl], ps[0:D, :], b_sb["bk"][0:D])
                        nc.vector.tensor_scalar_mul(
                            kz[0][D:P, tsl], ps[D:P, :], 0.0)
                        nc.vector.tensor_scalar_add(
                            kz[1][D:P, tsl], ps[D:P, :], b_sb["bk"][D:P])
                        nc.vector.tensor_scalar_mul(
                            kz[1][0:D, tsl], ps[0:D, :], 0.0)

            def vtl_block(sb):
                pst = psum.tile([P, P], F32R, tag="fctr", name="ps_tr",
                                bufs=1)
                nc.tensor.transpose(pst[:], vT[:, sb * P:(sb + 1) * P],
                                    identity[:])
                nc.vector.tensor_copy(out=vtl[:, sb, 0:D], in_=pst[:, 0:D])
                nc.vector.tensor_copy(out=vtl[:, sb, D + 1:2 * D + 1],
                                      in_=pst[:, D:2 * D])

            def attention_chunk(b, tcix):
                tsl = slice(b * T + tcix * TCH, b * T + (tcix + 1) * TCH)
                pv_ps = [
                    psum.tile([P, TCH], F32, tag=f"pv{h}",
                              name=f"ps_pv{h}", bufs=2)
                    for h in range(HPC)
                ]
                for sb in range(SBLK):
                    gsb = b * SBLK + sb
                    ssl = slice(b * T + sb * P, b * T + sb * P + P)
                    e_tiles = []
                    for h in range(HPC):
                        ps_st = psum.tile([P, TCH], F32, tag="mm",
                                          name="ps_st", bufs=3)
                        nc.tensor.matmul(ps_st[:], kz[h][:, ssl],
                                         qT[:, tsl], start=True, stop=True)
                        e = expp.tile([P, TCH], F32R, tag=f"e{h}",
                                      name=f"e{h}", bufs=4)
                        nc.scalar.activation(e[:], ps_st[:], Exp)
                        e_tiles.append(e)
                    for h in range(HPC):
                        last_pv = nc.tensor.matmul(
                            pv_ps[h][0:D + 1, :],
                            vtl[:, gsb, h * (D + 1):(h + 1) * (D + 1)],
                            e_tiles[h][:],
                            start=(sb == 0), stop=(sb == SBLK - 1))
                return pv_ps, last_pv

            def normalize_chunk(b, tcix, pv_ps):
                if b == 0:
                    dst, otsl = ag_in[0], slice(tcix * TCH, (tcix + 1) * TCH)
                elif tcix < 3:
                    dst, otsl = ag_in["1a"], slice(tcix * TCH,
                                                   (tcix + 1) * TCH)
                else:
                    dst, otsl = ag_in["1b"], slice(0, TCH)
                for h in range(HPC):
                    recip = work.tile([1, TCH], F32, tag="recip",
                                      name="recip", bufs=2)
                    nc.vector.reciprocal(recip[:], pv_ps[h][D:D + 1, :])
                    rb = dram.tile([1, TCH], F32, tag="recip_bounce",
                                   name="rb", bufs=2)
                    nc.sync.dma_start(rb[:], recip[:])
                    bc_sb = work.tile([D, TCH], F32, tag="bc",
                                      name="bc_sb", bufs=2)
                    nc.sync.dma_start(bc_sb[:],
                                      rb[0:1, :].to_broadcast([D, TCH]))
                    ot = work.tile([D, TCH], F32R, tag="ot", name="ot",
                                   bufs=2)
                    nc.vector.tensor_mul(out=ot[:], in0=pv_ps[h][0:D, :],
                                         in1=bc_sb[:])
                    nc.sync.dma_start(dst[h * D:(h + 1) * D, otsl], ot[:])

            def allgather(key):
                nc.gpsimd.collective_compute(
                    "AllGather", mybir.AluOpType.bypass,
                    replica_groups=[list(range(N_CORES))],
                    ins=[ag_in[key][:].opt()],
                    outs=[ag_out[key][:].opt()])

            def fc_chunk(b, tcix, after=None):
                tsl = slice(b * T + tcix * TCH, b * T + (tcix + 1) * TCH)
                if b == 0:
                    zsrc, otsl = ag_out[0], slice(tcix * TCH,
                                                  (tcix + 1) * TCH)
                elif tcix < 3:
                    zsrc, otsl = ag_out["1a"], slice(tcix * TCH,
                                                     (tcix + 1) * TCH)
                else:
                    zsrc, otsl = ag_out["1b"], slice(0, TCH)
                z_tiles = []
                for cb in range(CB):
                    z = work.tile([P, TCH], F32R, tag="z", name=f"z_{cb}",
                                  bufs=CB)
                    nc.sync.dma_start(
                        z[:], zsrc[cb * P:(cb + 1) * P, otsl])
                    z_tiles.append(z)
                ps = psum.tile([P, TCH], F32, tag="fctr", name="ps_fc",
                               bufs=1)
                for cb in range(CB):
                    mm = nc.tensor.matmul(ps[:], w_sb["wfc"][:, cb, :],
                                          z_tiles[cb],
                                          start=(cb == 0),
                                          stop=(cb == CB - 1))
                    if cb == 0 and after is not None:
                        add_dep_helper(mm.ins, after.ins, sync=False,
                                       reason="fc after attention chunk")
                osb = work.tile([P, TCH], F32, tag="osb", name="osb", bufs=2)
                nc.vector.tensor_scalar_add(osb[:], ps[:], b_sb["bfc"][:])
                nc.sync.dma_start(outT[:, tsl], osb[:])

            # emission order == scheduler priority. proj/FC matmuls act as
            # PE gap-fillers during the exp-bound attention phases.
            for tcix in range(NBC):
                proj_chunk(tcix)
            for sb in range(SBLK):
                vtl_block(sb)
            pending = None
            for tcix in range(NBC):
                pv0, _ = attention_chunk(0, tcix)
                if pending is not None:
                    normalize_chunk(0, pending[0], pending[1])
                pending = (tcix, pv0)
            normalize_chunk(0, pending[0], pending[1])
            allgather(0)
            for tcix in range(NBC, 2 * NBC):
                proj_chunk(tcix)
            for sb in range(SBLK, 2 * SBLK):
                vtl_block(sb)
            pending = None
            for tcix in range(NBC):
                pv1, last_pv = attention_chunk(1, tcix)
                if pending is not None:
                    normalize_chunk(1, pending[0], pending[1])
                    if pending[0] == 2:
                        allgather("1a")
                pending = (tcix, pv1)
                fc_chunk(0, tcix, after=last_pv)
            normalize_chunk(1, pending[0], pending[1])
            allgather("1b")
            for tcix in range(NBC):
                fc_chunk(1, tcix)

    split_sync_commands(nc)
    return nc


_CACHE = {}


def _prep_inputs(x, w_qkv, b_qkv, w_fc, b_fc):
    """Host-side: fold RoPE + scale into weights, shard per core."""
    rope = _rope_mats()
    w_qkv = np.asarray(w_qkv, dtype=np.float64)
    b_qkv = np.asarray(b_qkv, dtype=np.float64)
    wq_f = w_qkv[:, 0:C].copy()
    wk_f = w_qkv[:, C:2 * C].copy()
    wv_f = w_qkv[:, 2 * C:3 * C].copy()
    bq_f = b_qkv[0:C].copy()
    bk_f = b_qkv[C:2 * C].copy()
    bv_f = b_qkv[2 * C:3 * C].copy()
    scale = 1.0 / np.sqrt(D)
    for h in range(H):
        sl = slice(h * D, (h + 1) * D)
        wq_f[:, sl] = (wq_f[:, sl] @ rope[h]) * scale
        bq_f[sl] = (bq_f[sl] @ rope[h]) * scale
        wk_f[:, sl] = wk_f[:, sl] @ rope[h]
        bk_f[sl] = bk_f[sl] @ rope[h]

    xT = np.ascontiguousarray(
        np.asarray(x, dtype=np.float32).reshape(NT, C).T)

    in_maps = []
    for m in range(N_CORES):
        sl = slice(m * HB, (m + 1) * HB)
        in_maps.append({
            "xT": xT,
            "wq": np.ascontiguousarray(wq_f[:, sl], dtype=np.float32),
            "wk": np.ascontiguousarray(wk_f[:, sl], dtype=np.float32),
            "wv": np.ascontiguousarray(wv_f[:, sl], dtype=np.float32),
            "bq": np.ascontiguousarray(bq_f[sl, None], dtype=np.float32),
            "bk": np.ascontiguousarray(bk_f[sl, None], dtype=np.float32),
            "bv": np.ascontiguousarray(bv_f[sl, None], dtype=np.float32),
            "wfc": np.ascontiguousarray(w_fc[:, sl], dtype=np.float32),
            "bfc": np.ascontiguousarray(
                np.asarray(b_fc, dtype=np.float32)[sl, None]),
            "ident": np.eye(P, dtype=np.float32),
            "ones": np.ones((P, D), dtype=np.float32),
        })
    return in_maps


def kernel(x, w_qkv, b_qkv, w_fc, b_fc, _trace=False):
    in_maps = _prep_inputs(x, w_qkv, b_qkv, w_fc, b_fc)
    if "nc" not in _CACHE:
        _CACHE["nc"] = build_kernel()
    nc = _CACHE["nc"]
    res = run_bass_kernel_spmd(nc, in_maps, core_ids=list(range(N_CORES)),
                               trace=_trace)
    _CACHE["last_result"] = res
    out = np.concatenate(
        [res.results[m]["outT"].T for m in range(N_CORES)], axis=1)
    return np.ascontiguousarray(out.reshape(B, T, C))
